# revision 1
# baseline (speedup 1.0000x reference)
"""ChannelAttentionPropagation1D kernel for 8x TRN2 NeuronCores.

Reference computation (per batch b):
  kv[c,d]   = sum_{t,n} key_mem[b,t,n,c] * val_mem[b,t,n,d]    # (64, 64)
  kv_soft   = softmax(kv, axis=c)
  out[n,d]  = alpha * (key_cur[b] @ kv_soft)[n,d] + val_cur[b,n,d]

Sharding (8 cores):
  phase 1: core i contracts the t=i slice of key_mem/val_mem (16384 tokens
           per batch) into a partial kv^T, then AllReduce (64 KB) over cores.
  phase 2: core i computes the n-slice [2048*i, 2048*(i+1)) of the output.

Layout notes:
  - phase 1 accumulates kvT[d,c] (PSUM) so the softmax axis c lands on the
    free axis; a tiny PE transpose afterwards yields kv_soft[c,d].
  - key_cur is transposed (and scaled by alpha) on the host so its channel
    axis is the SBUF partition axis; its token axis is permuted n = 16p + j
    so phase-2 output tiles assemble into 4KB-contiguous-per-partition
    stores.
"""

import numpy as np

import concourse.bacc as bacc
import concourse.mybir as mybir
import concourse.tile as tile
from concourse import bass_utils, masks

F32 = mybir.dt.float32

N_CORES = 8
N, T, NTOK, C, C2 = 4, 8, 16384, 64, 64
NSL = NTOK // N_CORES  # 2048: phase-2 token slice per core
A_TILES = 64           # 128-token matmul tiles per half-batch chunk
HALF = NTOK // 2       # 8192 tokens per phase-1 DMA chunk

_CACHE = {}

# Extra kwargs forwarded to run_bass_kernel_spmd (used by the profiling
# harness to request an NTFF trace; empty for normal correctness runs).
_RUN_OPTS = {}


def _build_program():
    nc = bacc.Bacc(
        "TRN2",
        target_bir_lowering=False,
        debug=False,
        enable_asserts=False,
        num_devices=N_CORES,
    )

    km = nc.dram_tensor("key_mem", [N, NTOK, C], F32, kind="ExternalInput").ap()
    vm = nc.dram_tensor("val_mem", [N, NTOK, C2], F32, kind="ExternalInput").ap()
    # key_curT is host-packed [128, NSL/2]: rows 0:64 = channels for output
    # tiles j=0..7, rows 64:128 = channels for tiles j=8..15 (row-tiled
    # phase-2 pairs).
    kct = nc.dram_tensor(
        "key_curT", [N, 128, NSL // 2], F32, kind="ExternalInput"
    ).ap()
    vc = nc.dram_tensor("val_cur", [N, NSL, C2], F32, kind="ExternalInput").ap()
    out = nc.dram_tensor("out", [N, NSL, C2], F32, kind="ExternalOutput").ap()

    with tile.TileContext(nc) as tc:
        with (
            tc.tile_pool(name="persist", bufs=1) as persist,
            tc.tile_pool(name="big", bufs=4) as big,
            tc.tile_pool(name="tmp", bufs=2) as tmp,
            tc.tile_pool(name="stage", bufs=2) as stage_pool,
            tc.tile_pool(name="ps", bufs=2, space="PSUM") as ps,
            tc.tile_pool(name="dram", bufs=1, space="DRAM") as dram,
        ):
            ident = persist.tile([128, 128], F32)
            masks.make_identity(nc, ident[:])

            kct_sb = persist.tile([128, N * (NSL // 2)], F32)
            vc_sb = persist.tile([128, N * (NSL // 128) * C2], F32)

            kvt_sb = persist.tile([C2, N * C], F32)
            kvt_all = persist.tile([C2, N * N_CORES * C], F32)
            kvt_red = persist.tile([C2, N * C], F32)
            kv_soft = persist.tile([128, N * C2], F32)
            ar_outs = {}

            def emit_tails():
                """AR readbacks + softmax + transpose + phase 2 + stores for
                all batches, emitted STAGE-MAJOR: engine FIFOs run in program
                order, so batch-major emission would serialize the four
                ~15us-latency chains. Stage-major lets the four batches
                pipeline through gpsimd/DVE/ACT/PE. All tails sit after the
                whole phase-1 so a late AllReduce (peer-core launch skew can
                exceed 100us) never blocks local phase-1 work."""
                # readbacks ride the sync queue: its chunk DMAs have drained
                # by now, while gpsimd still holds doorbell-3 (which waits
                # for the end of phase-1) and scalar holds ar_in3. Each
                # AllGather result is [rank, d, c]; pull it into SBUF as
                # [d, (rank c)] and tree-reduce with 3 DVE adds per batch.
                W = N_CORES * C
                for b in range(N):
                    nc.sync.dma_start(
                        kvt_all[:, b * W:(b + 1) * W].rearrange(
                            "d (r c) -> d r c", r=N_CORES
                        ),
                        ar_outs[b].rearrange("r d c -> d r c"),
                    )
                for width in (4 * C, 2 * C):
                    for b in range(N):
                        lo = kvt_all[:, b * W: b * W + width]
                        nc.vector.tensor_add(
                            lo, lo, kvt_all[:, b * W + width: b * W + 2 * width]
                        )
                for b in range(N):
                    nc.vector.tensor_add(
                        kvt_red[:, b * C:(b + 1) * C],
                        kvt_all[:, b * W: b * W + C],
                        kvt_all[:, b * W + C: b * W + 2 * C],
                    )
                neg_mx = tmp.tile([C2, N], F32)
                for b in range(N):
                    nc.vector.reduce_max(
                        out=neg_mx[:, b:b + 1],
                        in_=kvt_red[:, b * C:(b + 1) * C],
                        axis=mybir.AxisListType.X,
                        negate=True,
                    )
                ex = tmp.tile([C2, N * C], F32)
                sm = tmp.tile([C2, N], F32)
                for b in range(N):
                    nc.scalar.activation(
                        ex[:, b * C:(b + 1) * C],
                        kvt_red[:, b * C:(b + 1) * C],
                        mybir.ActivationFunctionType.Exp,
                        bias=neg_mx[:, b:b + 1], scale=1.0,
                        accum_out=sm[:, b:b + 1],
                    )
                rv = tmp.tile([C2, N], F32)
                for b in range(N):
                    nc.vector.reciprocal(rv[:, b:b + 1], sm[:, b:b + 1])
                for b in range(N):
                    nc.vector.tensor_scalar_mul(
                        ex[:, b * C:(b + 1) * C],
                        ex[:, b * C:(b + 1) * C],
                        rv[:, b:b + 1],
                    )
                # Transpose softmaxed kvT to kv[c, d] (transpose-mode matmul
                # must write PSUM partition 0), then mirror the whole strip
                # into partitions 64:128 with one SBUF->SBUF DMA so row-tiled
                # phase-2 can read kv from the upper rows too.
                for b in range(N):
                    tp = ps.tile([C, C2], F32, tag="tp", name=f"tp{b}", bufs=2)
                    nc.tensor.transpose(
                        tp[:], ex[:, b * C:(b + 1) * C], ident[0:C2, 0:C2]
                    )
                    nc.vector.tensor_copy(
                        kv_soft[0:C, b * C2:(b + 1) * C2], tp[:]
                    )
                nc.sync.dma_start(kv_soft[64:64 + C, :], kv_soft[0:C, :])
                stgs = {}
                for b in range(N):
                    stgs[b] = stage_pool.tile(
                        [128, (NSL // 128) * C2], F32, tag=f"stg{b}",
                        name=f"stg{b}",
                    )
                # Row-tiled phase 2: tile j contracts on PE rows 0:64
                # (kct rows 0:64, kv rows 0:64), tile j+8 on rows 64:128 —
                # the two matmuls run concurrently on separate subarrays.
                HNSL = NSL // 2
                for b in range(N):
                    for j in range(8):
                        col = slice(b * HNSL + j * 128, b * HNSL + (j + 1) * 128)
                        o_a = ps.tile(
                            [128, C2], F32, tag="o", name=f"oa{b}_{j}", bufs=4
                        )
                        nc.tensor.matmul(
                            o_a[:],
                            lhsT=kct_sb[0:C, col],
                            rhs=kv_soft[0:C, b * C2:(b + 1) * C2],
                            start=True,
                            stop=True,
                            tile_position=(0, 0),
                        )
                        o_b = ps.tile(
                            [128, C2], F32, tag="o", name=f"ob{b}_{j}", bufs=4
                        )
                        nc.tensor.matmul(
                            o_b[:],
                            lhsT=kct_sb[64:64 + C, col],
                            rhs=kv_soft[64:64 + C, b * C2:(b + 1) * C2],
                            start=True,
                            stop=True,
                            tile_position=(64, 0),
                        )
                        nc.vector.tensor_add(
                            stgs[b][:, j * C2:(j + 1) * C2],
                            o_a[:],
                            vc_sb[:, b * 1024 + j * C2: b * 1024 + (j + 1) * C2],
                        )
                        nc.vector.tensor_add(
                            stgs[b][:, (j + 8) * C2:(j + 9) * C2],
                            o_b[:],
                            vc_sb[:, b * 1024 + (j + 8) * C2: b * 1024 + (j + 9) * C2],
                        )
                    # split the store so the second half overlaps the
                    # remaining adds (trims the last batch's tail)
                    oap = out[b].rearrange("(p j) c -> p (j c)", p=128)
                    nc.sync.dma_start(oap[:, 0:8 * C2], stgs[b][:, 0:8 * C2])
                    nc.sync.dma_start(
                        oap[:, 8 * C2:16 * C2], stgs[b][:, 8 * C2:16 * C2]
                    )

            # ---- phase 1: partial kvT[d, c] per batch, col-tiled 2x ----
            # Even token-tiles accumulate on PE column group 0 (psum rows
            # 0:64), odd tiles on column group 2 (psum rows 64:128); the two
            # halves' LDWEIGHTS/MATMUL overlap on independent subarrays.
            for b in range(N):
                kv_ps = ps.tile([128, C], F32, tag="kv", name=f"kv{b}")
                for h in range(2):
                    k_sb = big.tile([128, HALF // 128 * C], F32, tag="k")
                    v_sb = big.tile([128, HALF // 128 * C2], F32, tag="v")
                    sl = slice(h * HALF, (h + 1) * HALF)
                    nc.sync.dma_start(
                        k_sb[:], km[b, sl, :].rearrange("(p a) c -> p (a c)", p=128)
                    )
                    nc.sync.dma_start(
                        v_sb[:], vm[b, sl, :].rearrange("(p a) c -> p (a c)", p=128)
                    )
                    if h == 1:
                        # phase-2 inputs for batch b: issued on the scalar
                        # (ACT) DMA FIFO so they never delay the phase-1
                        # chunk stream on the sync FIFO.
                        nc.scalar.dma_start(
                            kct_sb[:, b * (NSL // 2):(b + 1) * (NSL // 2)],
                            kct[b],
                        )
                        nc.scalar.dma_start(
                            vc_sb[:, b * 1024:(b + 1) * 1024],
                            vc[b].rearrange("(p j) c -> p (j c)", p=128),
                        )
                    for a in range(A_TILES):
                        half = a % 2
                        nc.tensor.matmul(
                            kv_ps[64 * half:64 * half + C2, :],
                            lhsT=v_sb[:, a * C2:(a + 1) * C2],
                            rhs=k_sb[:, a * C:(a + 1) * C],
                            start=(h == 0 and a < 2),
                            stop=(h == 1 and a >= A_TILES - 2),
                            tile_position=(0, 64 * half),
                        )
                # partial kvT = even-half + odd-half (DVE can read only one
                # PSUM operand per instruction, so copy then add)
                nc.vector.tensor_copy(kvt_sb[:, b * C:(b + 1) * C], kv_ps[0:C2, :])
                nc.vector.tensor_add(
                    kvt_sb[:, b * C:(b + 1) * C],
                    kvt_sb[:, b * C:(b + 1) * C],
                    kv_ps[64:64 + C2, :],
                )
                # per-batch AllGather (cheaper than AllReduce on the CC
                # core); the 8 partials are tree-reduced locally on DVE.
                ar_in = dram.tile([C2, C], F32, tag=f"ar_in{b}", name=f"ar_in{b}")
                ar_out = dram.tile(
                    [N_CORES, C2, C], F32, addr_space="Shared", tag=f"ar_out{b}",
                    name=f"ar_out{b}",
                )
                ar_outs[b] = ar_out
                nc.scalar.dma_start(ar_in[:], kvt_sb[:, b * C:(b + 1) * C])
                nc.gpsimd.collective_compute(
                    "AllGather",
                    mybir.AluOpType.bypass,
                    replica_groups=[list(range(N_CORES))],
                    ins=[ar_in.opt()],
                    outs=[ar_out.opt()],
                )
            emit_tails()

    nc.compile()
    return nc


def _get_program():
    if "nc" not in _CACHE:
        _CACHE["nc"] = _build_program()
    return _CACHE["nc"]


def kernel(key_mem, val_mem, key_cur, val_cur, alpha):
    key_mem = np.asarray(key_mem, dtype=np.float32)
    val_mem = np.asarray(val_mem, dtype=np.float32)
    key_cur = np.asarray(key_cur, dtype=np.float32)
    val_cur = np.asarray(val_cur, dtype=np.float32)
    alpha_f = float(np.asarray(alpha).reshape(-1)[0])

    nc = _get_program()

    # key_cur^T with alpha folded in; token axis permuted so that SBUF
    # column j*128+p holds token p*16+j (phase-2 store contiguity).
    kc_scaled = (alpha_f * key_cur).astype(np.float32)
    in_maps = []
    for i in range(N_CORES):
        kct_i = kc_scaled[:, i * NSL:(i + 1) * NSL, :].transpose(0, 2, 1)
        kct_i = (
            kct_i.reshape(N, C, 128, NSL // 128)
            .transpose(0, 1, 3, 2)
            .reshape(N, C, NSL)
        )
        # pack for row-tiled phase 2: rows 0:64 = tiles j=0..7,
        # rows 64:128 = tiles j=8..15
        kct_i = (
            kct_i.reshape(N, C, 2, NSL // 2)
            .transpose(0, 2, 1, 3)
            .reshape(N, 128, NSL // 2)
        )
        in_maps.append(
            {
                "key_mem": np.ascontiguousarray(key_mem[:, i]),
                "val_mem": np.ascontiguousarray(val_mem[:, i]),
                "key_curT": np.ascontiguousarray(kct_i),
                "val_cur": np.ascontiguousarray(val_cur[:, i * NSL:(i + 1) * NSL, :]),
            }
        )

    res = bass_utils.run_bass_kernel_spmd(
        nc, in_maps, core_ids=list(range(N_CORES)), **_RUN_OPTS
    )
    _CACHE["last_result"] = res
    outs = [res.results[i]["out"] for i in range(N_CORES)]
    return np.concatenate(outs, axis=1).astype(np.float32)



# revision 4
# speedup vs baseline: 1.6520x; 1.6520x over previous
"""ChannelAttentionPropagation1D kernel for 8x TRN2 NeuronCores.

Reference computation (per batch b):
  kv[c,d]   = sum_{t,n} key_mem[b,t,n,c] * val_mem[b,t,n,d]    # (64, 64)
  kv_soft   = softmax(kv, axis=c)
  out[n,d]  = alpha * (key_cur[b] @ kv_soft)[n,d] + val_cur[b,n,d]

Sharding (8 cores): batch-pair. Core c owns batch b = c//2, token-half
h = c%2. Phase 1 contracts its 65536-token half of key_mem/val_mem into
a partial kvT; ONE 2-rank AllGather (cores 2b <-> 2b+1) exchanges the
16KB partials; both cores reduce + softmax locally. Phase 2 computes the
core's 8192-token slice of the output.

Precision: the kv softmax is extremely sharp (top-2 logit gap ~500) and
alpha is small, so fp16 inputs to both matmuls keep the final rel-fro
error ~2e-4, far under the 2e-2 gate, while halving HBM traffic. The
host casts key_mem/val_mem/key_curT/val_curT to fp16; PSUM accumulation
stays fp32 and the output is stored fp32.

Layouts:
  - phase 1 accumulates kvT[d,c] in PSUM with two alternating PE column
    groups so consecutive token-tiles overlap on independent subarrays.
  - phase 2 is computed TRANSPOSED: kv_soft[c,d] is the stationary PE
    operand and key_curT[c, tok] streams as N=512 moving tiles, giving
    outT[d, tok] (16 big matmuls instead of 128 small ones). Two row/col
    PE quadrants process the two 4096-token groups concurrently. The
    host transposes the returned [128, 4096] block back to [8192, 64].
"""

import numpy as np

import concourse.bacc as bacc
import concourse.mybir as mybir
import concourse.tile as tile
from concourse import bass_utils, masks

F32 = mybir.dt.float32
F16 = mybir.dt.float16

N_CORES = 8
N, T, NTOK, C, C2 = 4, 8, 16384, 64, 64
TOK_ALL = T * NTOK          # 131072 tokens per batch
TOK_HALF = TOK_ALL // 2     # 65536 phase-1 tokens per core
NSL = NTOK // 2             # 8192 phase-2 output tokens per core
CHUNK = 16384               # phase-1 tokens per DMA chunk
N_CHUNKS = TOK_HALF // CHUNK    # 4
A_TILES = CHUNK // 128      # 128 matmul token-tiles per chunk
G_TILES = 8                 # phase-2: 8 moving tiles of N=512 per quadrant

_CACHE = {}

# Extra kwargs forwarded to run_bass_kernel_spmd (used by the profiling
# harness to request an NTFF trace; empty for normal correctness runs).
_RUN_OPTS = {}


def _build_program():
    nc = bacc.Bacc(
        "TRN2",
        target_bir_lowering=False,
        debug=False,
        enable_asserts=False,
        num_devices=N_CORES,
    )

    km = nc.dram_tensor("key_mem", [TOK_HALF, C], F16, kind="ExternalInput").ap()
    vm = nc.dram_tensor("val_mem", [TOK_HALF, C2], F16, kind="ExternalInput").ap()
    # key_curT, val_curT: [128, 4096], rows 0:64 = channels for tokens
    # 0:4096 of the core's slice, rows 64:128 = channels for 4096:8192.
    # key_curT rows are c (alpha folded in); val_curT rows are d.
    kct = nc.dram_tensor("key_curT", [128, NSL // 2], F16, kind="ExternalInput").ap()
    vct = nc.dram_tensor("val_curT", [128, NSL // 2], F16, kind="ExternalInput").ap()
    # outT: rows 0:64 = out[d, tok] for tokens 0:4096, rows 64:128 for
    # tokens 4096:8192 (host transposes back).
    out = nc.dram_tensor("outT", [128, NSL // 2], F32, kind="ExternalOutput").ap()

    with tile.TileContext(nc) as tc:
        with (
            tc.tile_pool(name="persist", bufs=1) as persist,
            tc.tile_pool(name="big", bufs=3) as big,
            tc.tile_pool(name="tmp", bufs=1) as tmp,
            tc.tile_pool(name="ps", bufs=1, space="PSUM") as ps,
            tc.tile_pool(name="po", bufs=3, space="PSUM") as po,
            tc.tile_pool(name="dram", bufs=1, space="DRAM") as dram,
        ):
            ident = persist.tile([128, 128], F32)
            masks.make_identity(nc, ident[:])

            kct_sb = persist.tile([128, NSL // 2], F16)
            vct_sb = persist.tile([128, NSL // 2], F16)
            stg = persist.tile([128, NSL // 2], F32)

            kvt_sb = persist.tile([C2, C], F32)
            rb = persist.tile([C2, 2 * C], F32)
            kvt_red = persist.tile([C2, C], F32)
            kv_soft = persist.tile([128, C2], F16)

            # ---- phase 1: partial kvT[d, c], col-tiled 2x ----
            kv_ps = ps.tile([128, C], F32)
            for ch in range(N_CHUNKS):
                k_sb = big.tile([128, CHUNK // 128 * C], F16, tag="k")
                v_sb = big.tile([128, CHUNK // 128 * C2], F16, tag="v")
                sl = slice(ch * CHUNK, (ch + 1) * CHUNK)
                nc.sync.dma_start(
                    k_sb[:], km[sl, :].rearrange("(p a) c -> p (a c)", p=128)
                )
                nc.sync.dma_start(
                    v_sb[:], vm[sl, :].rearrange("(p a) c -> p (a c)", p=128)
                )
                if ch == 1:
                    # phase-2 inputs ride the scalar (ACT) DMA FIFO so they
                    # never delay the phase-1 chunk stream on the sync FIFO.
                    nc.scalar.dma_start(kct_sb[:], kct)
                    nc.scalar.dma_start(vct_sb[:], vct)
                for a in range(A_TILES):
                    half = a % 2
                    nc.tensor.matmul(
                        kv_ps[64 * half:64 * half + C2, :],
                        lhsT=v_sb[:, a * C2:(a + 1) * C2],
                        rhs=k_sb[:, a * C:(a + 1) * C],
                        start=(ch == 0 and a < 2),
                        stop=(ch == N_CHUNKS - 1 and a >= A_TILES - 2),
                        tile_position=(0, 64 * half),
                    )
            # partial kvT = even-half + odd-half (DVE reads only one PSUM
            # operand per instruction, so copy then add)
            nc.vector.tensor_copy(kvt_sb[:], kv_ps[0:C2, :])
            nc.vector.tensor_add(kvt_sb[:], kvt_sb[:], kv_ps[64:64 + C2, :])

            # ---- pairwise exchange: 2-rank AllGather of the 16KB partial
            ar_in = dram.tile([C2, C], F32, tag="ar_in", name="ar_in")
            ar_out = dram.tile([2, C2, C], F32, tag="ar_out", name="ar_out")
            nc.scalar.dma_start(ar_in[:], kvt_sb[:])
            nc.gpsimd.collective_compute(
                "AllGather",
                mybir.AluOpType.bypass,
                replica_groups=[[2 * i, 2 * i + 1] for i in range(4)],
                ins=[ar_in.opt()],
                outs=[ar_out.opt()],
            )
            nc.sync.dma_start(
                rb[:].rearrange("d (r c) -> d r c", r=2),
                ar_out.rearrange("r d c -> d r c"),
            )
            nc.vector.tensor_add(kvt_red[:], rb[:, 0:C], rb[:, C:2 * C])

            # ---- softmax over c (free axis) on kvT ----
            neg_mx = tmp.tile([C2, 1], F32)
            nc.vector.reduce_max(
                out=neg_mx[:],
                in_=kvt_red[:],
                axis=mybir.AxisListType.X,
                negate=True,
            )
            ex = tmp.tile([C2, C], F32)
            sm = tmp.tile([C2, 1], F32)
            nc.scalar.activation(
                ex[:],
                kvt_red[:],
                mybir.ActivationFunctionType.Exp,
                bias=neg_mx[:], scale=1.0,
                accum_out=sm[:],
            )
            rv = tmp.tile([C2, 1], F32)
            nc.vector.reciprocal(rv[:], sm[:])
            nc.vector.tensor_scalar_mul(ex[:], ex[:], rv[:])

            # transpose softmaxed kvT -> kv[c, d]; replicate into partitions
            # 64:128 (fp16 cast on the DVE copies) for the second quadrant.
            tp = ps.tile([C, C2], F32, tag="tp", name="tp")
            nc.tensor.transpose(tp[:], ex[:], ident[0:C2, 0:C2])
            nc.vector.tensor_copy(kv_soft[0:C, :], tp[:])
            nc.vector.tensor_copy(kv_soft[64:64 + C, :], tp[:])

            # ---- phase 2: outT[d, tok] = kv_soft[c,d]^T @ key_curT[c,tok]
            # Quadrant A (PE rows 0:64, cols 0:64) handles tokens 0:4096,
            # quadrant B (rows 64:128, cols 64:128) tokens 4096:8192.
            W = 512
            for g in range(G_TILES):
                o = po.tile([128, W], F32, tag="o", name=f"o{g}")
                col = slice(g * W, (g + 1) * W)
                nc.tensor.matmul(
                    o[0:C2, :],
                    lhsT=kv_soft[0:C, :],
                    rhs=kct_sb[0:C, col],
                    start=True, stop=True,
                    tile_position=(0, 0),
                )
                nc.tensor.matmul(
                    o[64:64 + C2, :],
                    lhsT=kv_soft[64:64 + C, :],
                    rhs=kct_sb[64:64 + C, col],
                    start=True, stop=True,
                    tile_position=(64, 64),
                )
                nc.vector.tensor_add(stg[:, col], o[:], vct_sb[:, col])
                if g % 2 == 1:
                    st = slice((g - 1) * W, (g + 1) * W)
                    nc.sync.dma_start(out[:, st], stg[:, st])

    nc.compile()
    return nc


def _get_program():
    if "nc" not in _CACHE:
        _CACHE["nc"] = _build_program()
    return _CACHE["nc"]


def kernel(key_mem, val_mem, key_cur, val_cur, alpha):
    key_mem = np.asarray(key_mem, dtype=np.float32)
    val_mem = np.asarray(val_mem, dtype=np.float32)
    key_cur = np.asarray(key_cur, dtype=np.float32)
    val_cur = np.asarray(val_cur, dtype=np.float32)
    alpha_f = float(np.asarray(alpha).reshape(-1)[0])

    nc = _get_program()

    km_flat = key_mem.reshape(N, TOK_ALL, C).astype(np.float16)
    vm_flat = val_mem.reshape(N, TOK_ALL, C2).astype(np.float16)
    kc_scaled = (alpha_f * key_cur).astype(np.float16)
    vc16 = val_cur.astype(np.float16)

    in_maps = []
    for core in range(N_CORES):
        b, h = divmod(core, 2)
        s0 = h * NSL
        kct_i = np.concatenate(
            [
                kc_scaled[b, s0:s0 + NSL // 2, :].T,
                kc_scaled[b, s0 + NSL // 2:s0 + NSL, :].T,
            ],
            axis=0,
        )
        vct_i = np.concatenate(
            [
                vc16[b, s0:s0 + NSL // 2, :].T,
                vc16[b, s0 + NSL // 2:s0 + NSL, :].T,
            ],
            axis=0,
        )
        in_maps.append(
            {
                "key_mem": np.ascontiguousarray(
                    km_flat[b, h * TOK_HALF:(h + 1) * TOK_HALF]
                ),
                "val_mem": np.ascontiguousarray(
                    vm_flat[b, h * TOK_HALF:(h + 1) * TOK_HALF]
                ),
                "key_curT": np.ascontiguousarray(kct_i),
                "val_curT": np.ascontiguousarray(vct_i),
            }
        )

    res = bass_utils.run_bass_kernel_spmd(
        nc, in_maps, core_ids=list(range(N_CORES)), **_RUN_OPTS
    )
    _CACHE["last_result"] = res

    out = np.empty((N, NTOK, C2), dtype=np.float32)
    for core in range(N_CORES):
        b, h = divmod(core, 2)
        s0 = h * NSL
        o = res.results[core]["outT"]
        out[b, s0:s0 + NSL // 2, :] = o[0:64, :].T
        out[b, s0 + NSL // 2:s0 + NSL, :] = o[64:128, :].T
    return out
